# revision 1
# baseline (speedup 1.0000x reference)
"""Trainium2 Bass kernel for the word2vec-style embedding_lookup problem.

reference math (per row b of data [B, 22], all f32):
  ctx_idx  = data[:, :10]    (into global_W [100001, 128])
  pos_idx  = data[:, 11]     (into sense_W  [300000, 128])
  neg_idx  = data[:, 12:17]  (into sense_W)
  mask     = data[:, 17:22]  (float multiplier for neg loss)
  ctx_feats = sum_j global_W[ctx_idx[:, j]] * ctx_weight[j]          # [B, 128]
  pos_ips   = dot(ctx_feats, sense_W[pos_idx])                        # [B]
  pos_loss  = sum(softplus(-clip(pos_ips, -10, 10)))
  neg_ips   = dot(ctx_feats, sense_W[neg_idx[:, n]])                  # [B, 5]
  neg_loss  = sum(softplus(clip(neg_ips, -10, 10)) * mask)

Sharding: data-parallel over 8 NeuronCores, 16384 rows each; the two
embedding tables are concatenated into one [400001, 128] table replicated
to every core.  Each core returns its two partial losses; the host sums.

Device mapping per 128-row block:
  - one gpsimd indirect DMA gathers the 16 embedding rows of each of the
    128 data rows into an SBUF tile [128, 16*128]
  - DVE: multiply ctx part by (pre-broadcast) ctx_weight, strided reduce
    over the 10 context slots, multiply sense part by broadcast ctx_feats
  - ACT: per-slot accumulate (dot products), then the clip+softplus chain
    via relu(x+10) -> relu(20-x) -> softplus(+/-(x-10))
  - per-block results land in slot buffers; one final reduce + PE
    ones-matmul collapses partitions to the two scalar losses.
"""

import numpy as np

V = 100000
D = 128
NCTX = 10  # 2*window
NSNS = 6   # 1 pos + 5 neg
K = NCTX + NSNS
B = 131072
NCORES = 8
BCORE = B // NCORES
NBLK_FULL = BCORE // 128
SENSE_OFF = V + 1
TABLE_ROWS_FULL = (V + 1) + 3 * V

_cache = {}


def build_nc(nblk, table_rows, debug_outs=False, slots_sched=None):
    """Build and compile the per-core Bass program.

    slots_sched: optional per-block gather count (11 + active negs); blocks
    gather only that many slots (trailing neg slots keep stale-but-finite
    data and are zeroed by their masks).  None -> 16 everywhere."""
    import concourse.bacc as bacc
    import concourse.bass as bass
    import concourse.mybir as mybir
    import concourse.tile as tile

    f32 = mybir.dt.float32
    i32 = mybir.dt.int32
    ALU = mybir.AluOpType
    ACTF = mybir.ActivationFunctionType
    AX = mybir.AxisListType

    nc = bacc.Bacc("TRN2", target_bir_lowering=False, debug=False)

    table = nc.dram_tensor("table", [table_rows, D], f32, kind="ExternalInput")
    idx = nc.dram_tensor("idx", [128, nblk * K], i32, kind="ExternalInput")
    msk = nc.dram_tensor("msk", [128, nblk * NSNS], f32, kind="ExternalInput")
    wb = nc.dram_tensor("wb", [128, NCTX * D], f32, kind="ExternalInput")
    out = nc.dram_tensor("out", [1, 2], f32, kind="ExternalOutput")
    if debug_outs:
        d_ips = nc.dram_tensor("d_ips", [128, nblk * NSNS], f32, kind="ExternalOutput")
        d_u = nc.dram_tensor("d_u", [128, nblk * NSNS], f32, kind="ExternalOutput")
        d_g = nc.dram_tensor("d_g", [128, K * D], f32, kind="ExternalOutput")
        d_F = nc.dram_tensor("d_F", [128, D], f32, kind="ExternalOutput")
        d_bufP = nc.dram_tensor("d_bufP", [128, nblk], f32, kind="ExternalOutput")
        d_bufN = nc.dram_tensor("d_bufN", [128, nblk * 5], f32, kind="ExternalOutput")

    with tile.TileContext(nc) as tc:
        with (
            tc.tile_pool(name="const", bufs=1) as constp,
            tc.tile_pool(name="gpool", bufs=4) as gp,
            tc.tile_pool(name="wpool", bufs=2) as wp,
            tc.tile_pool(name="spool", bufs=2) as sp,
            tc.tile_pool(name="small", bufs=2) as smp,
            tc.tile_pool(name="psum", bufs=1, space="PSUM") as psp,
        ):
            idx_t = constp.tile([128, nblk * K], i32)
            nc.sync.dma_start(out=idx_t[:], in_=idx[:])
            msk_t = constp.tile([128, nblk * NSNS], f32)
            nc.sync.dma_start(out=msk_t[:], in_=msk[:])
            wb_t = constp.tile([128, NCTX * D], f32)
            nc.sync.dma_start(out=wb_t[:], in_=wb[:])

            bufP = constp.tile([128, nblk], f32)
            bufN = constp.tile([128, nblk * 5], f32)
            dummy = constp.tile([128, D], f32)
            ones = constp.tile([128, 1], f32)
            nc.vector.memset(ones[:], 1.0)
            c10 = constp.tile([128, 1], f32)
            nc.vector.memset(c10[:], 10.0)
            c20 = constp.tile([128, 1], f32)
            nc.vector.memset(c20[:], 20.0)
            cm10 = constp.tile([128, 1], f32)
            nc.vector.memset(cm10[:], -10.0)

            for b in range(nblk):
                g = gp.tile([128, K * D], f32, tag="g")
                # HW vector-indirect DMA consumes ONE offset per partition
                # per instruction -> up to 16 gathers of [128, D] per block.
                nslots = K if slots_sched is None else slots_sched[b]
                for k in range(nslots):
                    nc.gpsimd.indirect_dma_start(
                        out=g[:, k * D : (k + 1) * D],
                        out_offset=None,
                        in_=table[:],
                        in_offset=bass.IndirectOffsetOnAxis(
                            ap=idx_t[:, b * K + k : b * K + k + 1], axis=0
                        ),
                    )
                # ctx part * ctx_weight
                wprod = wp.tile([128, NCTX * D], f32, tag="wprod")
                nc.vector.tensor_tensor(
                    out=wprod[:], in0=g[:, : NCTX * D], in1=wb_t[:], op=ALU.mult
                )
                # ctx_feats: reduce over the 10 ctx slots (strided view)
                F = smp.tile([128, D], f32, tag="F")
                nc.vector.tensor_reduce(
                    out=F[:],
                    in_=wprod[:].rearrange("p (j d) -> p d j", j=NCTX),
                    axis=AX.X,
                    op=ALU.add,
                )
                # sense part * broadcast ctx_feats
                S = sp.tile([128, NSNS * D], f32, tag="S")
                nc.vector.tensor_tensor(
                    out=S[:].rearrange("p (n d) -> p n d", n=NSNS),
                    in0=g[:, NCTX * D :].rearrange("p (n d) -> p n d", n=NSNS),
                    in1=F[:].unsqueeze(1).to_broadcast([128, NSNS, D]),
                    op=ALU.mult,
                )
                # dot products: per-slot free-dim accumulate on ACT
                ips = smp.tile([128, NSNS], f32, tag="ips")
                for n in range(NSNS):
                    nc.scalar.activation(
                        out=dummy[:],
                        in_=S[:, n * D : (n + 1) * D],
                        func=ACTF.Copy,
                        accum_out=ips[:, n : n + 1],
                    )
                # clip+softplus chain:
                #   t = relu(ips + 10); u = relu(20 - t)  (u = 10 - clip(ips))
                #   pos elem = softplus(u0 - 10);  neg elem = softplus(10 - u)
                t1 = smp.tile([128, NSNS], f32, tag="t1")
                nc.scalar.activation(
                    out=t1[:], in_=ips[:], func=ACTF.Relu, bias=c10[:], scale=1.0
                )
                u = smp.tile([128, NSNS], f32, tag="u")
                nc.scalar.activation(
                    out=u[:], in_=t1[:], func=ACTF.Relu, bias=c20[:], scale=-1.0
                )
                # softplus(x) = Ln(exp(x) + 1); pos x = u0 - 10, neg x = 10 - u
                ep = smp.tile([128, 1], f32, tag="ep")
                nc.scalar.activation(
                    out=ep[:], in_=u[:, 0:1], func=ACTF.Exp, bias=cm10[:], scale=1.0
                )
                nc.scalar.activation(
                    out=bufP[:, b : b + 1], in_=ep[:], func=ACTF.Ln, bias=1.0, scale=1.0
                )
                en = smp.tile([128, 5], f32, tag="en")
                nc.scalar.activation(
                    out=en[:], in_=u[:, 1:NSNS], func=ACTF.Exp, bias=c10[:], scale=-1.0
                )
                Ln = smp.tile([128, 5], f32, tag="Ln")
                nc.scalar.activation(
                    out=Ln[:], in_=en[:], func=ACTF.Ln, bias=1.0, scale=1.0
                )
                nc.vector.tensor_tensor(
                    out=bufN[:, b * 5 : (b + 1) * 5],
                    in0=Ln[:],
                    in1=msk_t[:, b * NSNS + 1 : (b + 1) * NSNS],
                    op=ALU.mult,
                )
                if debug_outs:
                    nc.sync.dma_start(
                        out=d_ips[:, b * NSNS : (b + 1) * NSNS], in_=ips[:]
                    )
                    nc.sync.dma_start(out=d_u[:, b * NSNS : (b + 1) * NSNS], in_=u[:])
                    if b == 0:
                        nc.sync.dma_start(out=d_g[:], in_=g[:])
                        nc.sync.dma_start(out=d_F[:], in_=F[:])

            if debug_outs:
                nc.sync.dma_start(out=d_bufP[:], in_=bufP[:])
                nc.sync.dma_start(out=d_bufN[:], in_=bufN[:])
            acc2 = constp.tile([128, 2], f32)
            nc.vector.tensor_reduce(
                out=acc2[:, 0:1], in_=bufP[:], axis=AX.X, op=ALU.add
            )
            nc.vector.tensor_reduce(
                out=acc2[:, 1:2], in_=bufN[:], axis=AX.X, op=ALU.add
            )
            ps = psp.tile([1, 2], f32)
            nc.tensor.matmul(out=ps[:], lhsT=ones[:], rhs=acc2[:], start=True, stop=True)
            fin = smp.tile([1, 2], f32, tag="fin")
            nc.vector.tensor_copy(out=fin[:], in_=ps[:])
            nc.sync.dma_start(out=out[:], in_=fin[:])

    nc.compile()
    return nc


def get_nc(nblk, table_rows, slots_sched=None):
    key = (nblk, table_rows, slots_sched)
    if key not in _cache:
        _cache[key] = build_nc(nblk, table_rows, slots_sched=slots_sched)
    return _cache[key]


def host_prep(data, global_W, sense_W, ctx_weight, ncores, nblk):
    """Shard + lay out the inputs for the per-core kernel."""
    data = np.asarray(data)
    global_W = np.asarray(global_W, dtype=np.float32)
    sense_W = np.asarray(sense_W, dtype=np.float32)
    ctx_weight = np.asarray(ctx_weight, dtype=np.float32)

    b = data.shape[0]
    bcore = b // ncores
    assert bcore == nblk * 128

    idx_all = np.empty((b, K), dtype=np.int32)
    idx_all[:, :NCTX] = data[:, :NCTX]
    idx_all[:, NCTX] = data[:, NCTX + 1] + SENSE_OFF
    idx_all[:, NCTX + 1 :] = data[:, NCTX + 2 : NCTX + 7] + SENSE_OFF

    msk_all = np.empty((b, NSNS), dtype=np.float32)
    msk_all[:, 0] = 1.0
    msk_all[:, 1:] = data[:, NCTX + 7 :].astype(np.float32)

    table = np.ascontiguousarray(
        np.concatenate([global_W, sense_W], axis=0), dtype=np.float32
    )
    wb = np.ascontiguousarray(
        np.broadcast_to(ctx_weight.reshape(1, NCTX * D), (128, NCTX * D)),
        dtype=np.float32,
    )

    in_maps = []
    for c in range(ncores):
        sl = slice(c * bcore, (c + 1) * bcore)
        idx_c = np.ascontiguousarray(
            idx_all[sl].reshape(nblk, 128, K).transpose(1, 0, 2).reshape(128, nblk * K)
        )
        msk_c = np.ascontiguousarray(
            msk_all[sl]
            .reshape(nblk, 128, NSNS)
            .transpose(1, 0, 2)
            .reshape(128, nblk * NSNS)
        )
        in_maps.append({"table": table, "idx": idx_c, "msk": msk_c, "wb": wb})
    return in_maps


# Mask-aware gather elision (v3): neg slots are symmetric, so compact each
# row's active negs to the front, sort rows by active count (descending), and
# gather only 11 + active slots per block.  The per-block schedule must be
# compile-time; SLOTS_SCHED is the binomial(5,1/2) quantile schedule with
# boundaries shifted one block late for safety.  Overflow -> plain v1.
def make_slots_sched(nblk, margin=1):
    """Binomial(5,1/2) quantile schedule; boundaries shifted `margin` blocks
    late (margin=1 -> 1733 insts, ~+2.6 sigma; margin=2 -> 1743, ~+5 sigma)."""
    bounds = (4, 24, 64, 104, 124)
    sched = []
    for i in range(nblk):
        c = 5
        for ci, bnd in enumerate(bounds):
            if i > bnd + margin:
                c = 4 - ci
        sched.append(11 + max(c, 0))
    return tuple(sched)


def host_prep_v3(data, global_W, sense_W, ctx_weight, ncores, nblk, sched):
    """v1 layout + neg compaction + per-core row sort by active-neg count.
    Returns (in_maps, ok); ok=False when a block exceeds its scheduled slots."""
    data = np.asarray(data)
    b = data.shape[0]
    bcore = b // ncores

    idx_all = np.empty((b, K), dtype=np.int32)
    idx_all[:, :NCTX] = data[:, :NCTX]
    idx_all[:, NCTX] = data[:, NCTX + 1] + SENSE_OFF
    neg = np.asarray(data[:, NCTX + 2 : NCTX + 7], dtype=np.int32)
    mask = np.asarray(data[:, NCTX + 7 :])
    act = mask != 0
    # compact active negs to the front (stable), masks follow
    ordn = np.argsort(~act, axis=1, kind="stable")
    rowi = np.arange(b)[:, None]
    idx_all[:, NCTX + 1 :] = neg[rowi, ordn] + SENSE_OFF
    msk_all = np.empty((b, NSNS), dtype=np.float32)
    msk_all[:, 0] = 1.0
    msk_all[:, 1:] = mask[rowi, ordn].astype(np.float32)
    cnt = act.sum(axis=1)

    table = np.ascontiguousarray(
        np.concatenate(
            [np.asarray(global_W, np.float32), np.asarray(sense_W, np.float32)],
            axis=0,
        )
    )
    wb = np.ascontiguousarray(
        np.broadcast_to(
            np.asarray(ctx_weight, np.float32).reshape(1, NCTX * D),
            (128, NCTX * D),
        )
    )

    in_maps = []
    for c in range(ncores):
        sl = slice(c * bcore, (c + 1) * bcore)
        order = np.argsort(-cnt[sl], kind="stable")
        csort = cnt[sl][order]
        # schedule feasibility: every row's count within its block's budget
        blockmax = csort.reshape(nblk, 128).max(axis=1)
        if any(blockmax[i] > sched[i] - 11 for i in range(nblk)):
            return None, False
        idx_c = np.ascontiguousarray(
            idx_all[sl][order]
            .reshape(nblk, 128, K)
            .transpose(1, 0, 2)
            .reshape(128, nblk * K)
        )
        msk_c = np.ascontiguousarray(
            msk_all[sl][order]
            .reshape(nblk, 128, NSNS)
            .transpose(1, 0, 2)
            .reshape(128, nblk * NSNS)
        )
        in_maps.append({"table": table, "idx": idx_c, "msk": msk_c, "wb": wb})
    return in_maps, True


# ---------------------------------------------------------------------------
# v2: two-stage gather via dma_gather (fast path)
#
# Per superblock of SBB=16 blocks (2048 data rows, 32768 canonical lookup
# positions laid out pos = b2*2048 + k*128 + p):
#   stage 1: lookups bucketed by fixed 32768-row table segments; each segment
#     has a compile-time capacity window in a DRAM staging buffer.  dma_gather
#     (int16 idx relative to the segment base, padded with dummy row 0) pulls
#     each bucket into SBUF; an HWDGE DMA drains it to the staging window.
#   stage 2: per 128-row block, dma_gather from the staging buffer using the
#     int16 staging position of each canonical slot (masked-out neg slots
#     reuse the row's pos slot so every slot is written with real data).
# Compute per block is identical to v1.
# ---------------------------------------------------------------------------

SEGW = 32768  # table rows per segment (int16 index space)
SBB = 16      # blocks per superblock
SBROWS = SBB * 128          # 2048 data rows
SBPOS = SBROWS * K          # 32768 canonical positions
# capacities (multiples of 128) per segment for the reference distribution:
# ctx uniform over [0, 100001), pos/neg over [100001, 200001), ~half the neg
# slots masked out.  Segments past row 229376 are never touched.
CAPS_FULL = (7424, 7424, 7424, 2944, 2688, 2688, 384)
MAXCALL = 2048


def plan_calls(caps):
    """[(seg, seg_base_row, stage_pos_base, call_len), ...] and total rows."""
    calls = []
    pos = 0
    for s, cap in enumerate(caps):
        left = cap
        cbase = pos
        while left > 0:
            l = min(MAXCALL, left)
            calls.append((s, s * SEGW, cbase, l))
            cbase += l
            left -= l
        pos += cap
    return calls, pos


def build_nc_v2(nblk, table_rows, caps, dt16=False):
    import concourse.bacc as bacc
    import concourse.mybir as mybir
    import concourse.tile as tile
    from concourse.library_config import mlp

    f32 = mybir.dt.float32
    i16 = mybir.dt.int16
    gdt = mybir.dt.bfloat16 if dt16 else f32
    ALU = mybir.AluOpType
    ACTF = mybir.ActivationFunctionType
    AX = mybir.AxisListType

    nsb = nblk // SBB
    calls, stage_rows = plan_calls(caps)
    i1cols = sum(l // 16 for (_, _, _, l) in calls)
    i2cols = SBPOS // 16

    nc = bacc.Bacc("TRN2", target_bir_lowering=False, debug=False)
    table = nc.dram_tensor("table", [table_rows, D], gdt, kind="ExternalInput")
    idx1 = nc.dram_tensor("idx1", [128, nsb * i1cols], i16, kind="ExternalInput")
    idx2 = nc.dram_tensor("idx2", [128, nsb * i2cols], i16, kind="ExternalInput")
    msk = nc.dram_tensor("msk", [128, nblk * NSNS], f32, kind="ExternalInput")
    wb = nc.dram_tensor("wb", [128, NCTX * D], gdt, kind="ExternalInput")
    out = nc.dram_tensor("out", [1, 2], f32, kind="ExternalOutput")

    with tile.TileContext(nc) as tc:
        with (
            tc.tile_pool(name="const", bufs=1) as constp,
            tc.tile_pool(name="ipool", bufs=2) as ip,
            tc.tile_pool(name="s1pool", bufs=4) as s1p,
            tc.tile_pool(name="dram", bufs=2, space="DRAM") as dp,
            tc.tile_pool(name="gpool", bufs=3) as gp,
            tc.tile_pool(name="wpool", bufs=2) as wp,
            tc.tile_pool(name="spool", bufs=2) as sp,
            tc.tile_pool(name="small", bufs=2) as smp,
            tc.tile_pool(name="psum", bufs=1, space="PSUM") as psp,
        ):
            nc.gpsimd.load_library(mlp)
            msk_t = constp.tile([128, nblk * NSNS], f32)
            nc.sync.dma_start(out=msk_t[:], in_=msk[:])
            wb_t = constp.tile([128, NCTX * D], gdt)
            nc.sync.dma_start(out=wb_t[:], in_=wb[:])

            bufP = constp.tile([128, nblk], f32)
            bufN = constp.tile([128, nblk * 5], f32)
            dummy = constp.tile([128, D], f32)
            ones = constp.tile([128, 1], f32)
            nc.vector.memset(ones[:], 1.0)
            c10 = constp.tile([128, 1], f32)
            nc.vector.memset(c10[:], 10.0)
            c20 = constp.tile([128, 1], f32)
            nc.vector.memset(c20[:], 20.0)
            cm10 = constp.tile([128, 1], f32)
            nc.vector.memset(cm10[:], -10.0)

            for sb in range(nsb):
                idx1_t = ip.tile([128, i1cols], i16, tag="i1")
                nc.sync.dma_start(
                    out=idx1_t[:], in_=idx1[:, sb * i1cols : (sb + 1) * i1cols]
                )
                idx2_t = ip.tile([128, i2cols], i16, tag="i2")
                nc.sync.dma_start(
                    out=idx2_t[:], in_=idx2[:, sb * i2cols : (sb + 1) * i2cols]
                )
                staged = dp.tile([stage_rows, D], gdt, tag="staged")
                for (s, base, cbase, l) in calls:
                    s1 = s1p.tile([128, (MAXCALL // 128) * D], gdt, tag="s1")
                    nc.gpsimd.dma_gather(
                        s1[:, : (l // 128) * D].rearrange(
                            "p (c d) -> p c d", c=l // 128
                        ),
                        table[base:, :],
                        idx1_t[:, cbase // 16 : cbase // 16 + l // 16],
                        l,
                        l,
                        D,
                        single_packet=False,
                    )
                    nc.sync.dma_start(
                        out=staged[cbase : cbase + l, :].rearrange(
                            "(c p) d -> p c d", p=128
                        ),
                        in_=s1[:, : (l // 128) * D].rearrange(
                            "p (c d) -> p c d", c=l // 128
                        ),
                    )
                for b2 in range(SBB):
                    b = sb * SBB + b2
                    g = gp.tile([128, K * D], gdt, tag="g")
                    nc.gpsimd.dma_gather(
                        g[:].rearrange("p (n d) -> p n d", n=K),
                        staged[:, :],
                        idx2_t[:, b2 * 128 : (b2 + 1) * 128],
                        SBPOS // SBB,
                        SBPOS // SBB,
                        D,
                        single_packet=False,
                    )
                    wprod = wp.tile([128, NCTX * D], gdt, tag="wprod")
                    nc.vector.tensor_tensor(
                        out=wprod[:], in0=g[:, : NCTX * D], in1=wb_t[:], op=ALU.mult
                    )
                    F = smp.tile([128, D], f32, tag="F")
                    nc.vector.tensor_reduce(
                        out=F[:],
                        in_=wprod[:].rearrange("p (j d) -> p d j", j=NCTX),
                        axis=AX.X,
                        op=ALU.add,
                    )
                    if dt16:
                        Fb = smp.tile([128, D], gdt, tag="Fb")
                        nc.vector.tensor_copy(out=Fb[:], in_=F[:])
                    else:
                        Fb = F
                    S = sp.tile([128, NSNS * D], gdt, tag="S")
                    nc.vector.tensor_tensor(
                        out=S[:].rearrange("p (n d) -> p n d", n=NSNS),
                        in0=g[:, NCTX * D :].rearrange("p (n d) -> p n d", n=NSNS),
                        in1=Fb[:].unsqueeze(1).to_broadcast([128, NSNS, D]),
                        op=ALU.mult,
                    )
                    ips = smp.tile([128, NSNS], f32, tag="ips")
                    for n in range(NSNS):
                        nc.scalar.activation(
                            out=dummy[:],
                            in_=S[:, n * D : (n + 1) * D],
                            func=ACTF.Copy,
                            accum_out=ips[:, n : n + 1],
                        )
                    t1 = smp.tile([128, NSNS], f32, tag="t1")
                    nc.scalar.activation(
                        out=t1[:], in_=ips[:], func=ACTF.Relu, bias=c10[:], scale=1.0
                    )
                    u = smp.tile([128, NSNS], f32, tag="u")
                    nc.scalar.activation(
                        out=u[:], in_=t1[:], func=ACTF.Relu, bias=c20[:], scale=-1.0
                    )
                    ep = smp.tile([128, 1], f32, tag="ep")
                    nc.scalar.activation(
                        out=ep[:], in_=u[:, 0:1], func=ACTF.Exp, bias=cm10[:], scale=1.0
                    )
                    nc.scalar.activation(
                        out=bufP[:, b : b + 1], in_=ep[:], func=ACTF.Ln,
                        bias=1.0, scale=1.0,
                    )
                    en = smp.tile([128, 5], f32, tag="en")
                    nc.scalar.activation(
                        out=en[:], in_=u[:, 1:NSNS], func=ACTF.Exp,
                        bias=c10[:], scale=-1.0,
                    )
                    Ln = smp.tile([128, 5], f32, tag="Ln")
                    nc.scalar.activation(
                        out=Ln[:], in_=en[:], func=ACTF.Ln, bias=1.0, scale=1.0
                    )
                    nc.vector.tensor_tensor(
                        out=bufN[:, b * 5 : (b + 1) * 5],
                        in0=Ln[:],
                        in1=msk_t[:, b * NSNS + 1 : (b + 1) * NSNS],
                        op=ALU.mult,
                    )

            acc2 = constp.tile([128, 2], f32)
            nc.vector.tensor_reduce(
                out=acc2[:, 0:1], in_=bufP[:], axis=AX.X, op=ALU.add
            )
            nc.vector.tensor_reduce(
                out=acc2[:, 1:2], in_=bufN[:], axis=AX.X, op=ALU.add
            )
            ps = psp.tile([1, 2], f32)
            nc.tensor.matmul(out=ps[:], lhsT=ones[:], rhs=acc2[:], start=True, stop=True)
            fin = smp.tile([1, 2], f32, tag="fin")
            nc.vector.tensor_copy(out=fin[:], in_=ps[:])
            nc.sync.dma_start(out=out[:], in_=fin[:])

    nc.compile()
    return nc


def get_nc_v2(nblk, table_rows, caps, dt16=False):
    key = ("v2", nblk, table_rows, caps, dt16)
    if key not in _cache:
        _cache[key] = build_nc_v2(nblk, table_rows, caps, dt16)
    return _cache[key]


def _wrap16(lst):
    """index list (len mult of 16) -> [128, len/16] int16 (16-partition wrap,
    replicated into the 8 groups of 16 partitions)."""
    a = np.asarray(lst, dtype=np.int16).reshape(-1, 16).T  # [16, cols]
    return np.tile(a, (8, 1))


def host_prep_v2(data, global_W, sense_W, ctx_weight, ncores, nblk, caps,
                 dt16=False):
    """Returns (in_maps, ok).  ok=False -> a segment overflowed its capacity
    (input distribution differs from expectations); caller falls back to v1."""
    data = np.asarray(data)
    global_W = np.asarray(global_W, dtype=np.float32)
    sense_W = np.asarray(sense_W, dtype=np.float32)
    ctx_weight = np.asarray(ctx_weight, dtype=np.float32)

    b = data.shape[0]
    bcore = b // ncores
    nsb = nblk // SBB
    calls, stage_rows = plan_calls(caps)
    nseg = len(caps)
    segpos = np.concatenate([[0], np.cumsum(caps)]).astype(np.int64)
    if stage_rows > 32768:
        return None, False

    idx_all = np.empty((b, K), dtype=np.int64)
    idx_all[:, :NCTX] = data[:, :NCTX]
    idx_all[:, NCTX] = data[:, NCTX + 1] + SENSE_OFF
    idx_all[:, NCTX + 1 :] = data[:, NCTX + 2 : NCTX + 7] + SENSE_OFF
    if idx_all.max() >= nseg * SEGW:
        return None, False

    maskv = np.ones((b, K), dtype=bool)
    maskv[:, NCTX + 1 :] = data[:, NCTX + 7 :] != 0

    msk_all = np.empty((b, NSNS), dtype=np.float32)
    msk_all[:, 0] = 1.0
    msk_all[:, 1:] = data[:, NCTX + 7 :].astype(np.float32)

    import ml_dtypes

    hdt = ml_dtypes.bfloat16 if dt16 else np.float32
    table = np.ascontiguousarray(
        np.concatenate([global_W, sense_W], axis=0).astype(hdt)
    )
    wb = np.ascontiguousarray(
        np.broadcast_to(ctx_weight.reshape(1, NCTX * D), (128, NCTX * D))
    ).astype(hdt)

    in_maps = []
    for c in range(ncores):
        sl = slice(c * bcore, (c + 1) * bcore)
        msk_c = np.ascontiguousarray(
            msk_all[sl]
            .reshape(nblk, 128, NSNS)
            .transpose(1, 0, 2)
            .reshape(128, nblk * NSNS)
        )
        i1_parts, i2_parts = [], []
        for sb in range(nsb):
            r0 = c * bcore + sb * SBROWS
            tbl = idx_all[r0 : r0 + SBROWS]  # [2048, 16]
            vld = maskv[r0 : r0 + SBROWS]
            # canonical order [b2, k, p]
            tbl_f = tbl.reshape(SBB, 128, K).transpose(0, 2, 1).reshape(-1)
            vld_f = vld.reshape(SBB, 128, K).transpose(0, 2, 1).reshape(-1)
            vpos = np.nonzero(vld_f)[0]
            rows = tbl_f[vpos]
            order = np.argsort(rows, kind="stable")
            srows = rows[order]
            seg = (srows // SEGW).astype(np.int64)
            counts = np.bincount(seg, minlength=nseg)
            if (counts > np.asarray(caps)).any():
                return None, False
            # stage positions in sorted order
            seg_starts = np.concatenate([[0], np.cumsum(counts)])[:-1]
            rank = np.arange(len(srows)) - seg_starts[seg]
            spos_sorted = segpos[seg] + rank
            # stage-1 index array (padded with dummy 0 per segment window)
            idx1_flat = np.zeros(stage_rows, dtype=np.int64)
            idx1_flat[spos_sorted] = srows - seg * SEGW
            # stage-2: canonical position -> stage position
            sp_f = np.zeros(SBPOS, dtype=np.int64)
            sp_f[vpos[order]] = spos_sorted
            # masked slots reuse the row's pos slot (k=NCTX)
            posslot = (
                (np.arange(SBPOS) // (K * 128)) * (K * 128)
                + NCTX * 128
                + (np.arange(SBPOS) % 128)
            )
            sp_f = np.where(vld_f, sp_f, sp_f[posslot])
            i1_parts.append(_wrap16(idx1_flat))
            i2_parts.append(_wrap16(sp_f))
        in_maps.append(
            {
                "table": table,
                "idx1": np.ascontiguousarray(np.concatenate(i1_parts, axis=1)),
                "idx2": np.ascontiguousarray(np.concatenate(i2_parts, axis=1)),
                "msk": msk_c,
                "wb": wb,
            }
        )
    return in_maps, True


def kernel(data, global_W, sense_W, ctx_weight, window, negative):
    # v3: per-slot vector-indirect gathers with mask-aware elision — rows
    # sorted by active-neg count (loss is order-invariant), active negs
    # compacted to the front, and each block gathers only 11+active slots
    # (~1733 instead of 2048 Pool instructions).  The kernel is Pool/SWDGE
    # descriptor-generation bound, so this directly cuts the critical path;
    # falls back to the fixed 16-slot v1 schedule if the mask distribution
    # overflows the compile-time schedule.  (A two-stage dma_gather pipeline
    # and a bf16 variant both measured slower: descriptor-rate-bound.)
    from concourse.bass_utils import run_bass_kernel_spmd

    assert int(window) == 5 and int(negative) == 5

    nc = None
    for margin in (1, 2):
        sched = make_slots_sched(NBLK_FULL, margin)
        in_maps, ok = host_prep_v3(
            data, global_W, sense_W, ctx_weight, NCORES, NBLK_FULL, sched
        )
        if ok:
            nc = get_nc(NBLK_FULL, TABLE_ROWS_FULL, slots_sched=sched)
            break
    if nc is None:
        nc = get_nc(NBLK_FULL, TABLE_ROWS_FULL)
        in_maps = host_prep(data, global_W, sense_W, ctx_weight, NCORES, NBLK_FULL)
    res = run_bass_kernel_spmd(nc, in_maps, core_ids=list(range(NCORES)))
    outs = np.stack([r["out"][0] for r in res.results])  # [ncores, 2]
    tot = outs.sum(axis=0)
    return (np.float32(tot[0]), np.float32(tot[1]))



# revision 2
# speedup vs baseline: 1.4721x; 1.4721x over previous
"""Trainium2 Bass kernel for the word2vec-style embedding_lookup problem.

reference math (per row b of data [B, 22], all f32):
  ctx_idx  = data[:, :10]    (into global_W [100001, 128])
  pos_idx  = data[:, 11]     (into sense_W  [300000, 128])
  neg_idx  = data[:, 12:17]  (into sense_W)
  mask     = data[:, 17:22]  (float multiplier for neg loss)
  ctx_feats = sum_j global_W[ctx_idx[:, j]] * ctx_weight[j]          # [B, 128]
  pos_ips   = dot(ctx_feats, sense_W[pos_idx])                        # [B]
  pos_loss  = sum(softplus(-clip(pos_ips, -10, 10)))
  neg_ips   = dot(ctx_feats, sense_W[neg_idx[:, n]])                  # [B, 5]
  neg_loss  = sum(softplus(clip(neg_ips, -10, 10)) * mask)

Sharding: data-parallel over 8 NeuronCores, 16384 rows each; the two
embedding tables are concatenated into one [400001, 128] table replicated
to every core.  Each core returns its two partial losses; the host sums.

v4 gather engine (the big win over per-slot vector-indirect DMA):
  The HW's vector-indirect DMA consumes one offset per partition per
  instruction (128 rows / ~1.5us of serialized Pool-engine time), which
  made the baseline descriptor-generation bound at ~44 GB/s.  dma_gather
  amortizes descriptor generation over thousands of rows per instruction,
  and round-robining calls over the 4 SWDGE queues (num_swdge_queues=4)
  parallelizes the descriptor pipeline: measured ~331 GB/s sustained on
  random 512B row gathers (vs ~73 GB/s on one queue).

  dma_gather only takes int16 indices, so a direct canonical-order gather
  of a 400001-row table is impossible.  Two stages per superblock of 16
  blocks (2048 data rows, 32768 canonical slot positions):
    stage 1: the valid lookups are deduplicated and sorted by table row;
      fixed 32768-row table segments get data-derived capacity windows in
      a <=32768-row DRAM staging buffer.  dma_gather calls (RR queues)
      pull each window into SBUF and one contiguous-per-partition HWDGE
      DMA per window drains it to staging.
    stage 2: per 128-row block, one dma_gather (2048 int16 staging rows)
      rebuilds the canonical [128, 16*D] tile; masked-out neg slots point
      at the row's pos slot and are zeroed by the mask multiply.
  The gather pool is 8 tiles deep -- shallower pipelines expose the
  ~6-8us gather latency and stall the DVE stream.

Compute per block (unchanged from the baseline):
  DVE: ctx part * ctx_weight, strided reduce over the 10 context slots,
  sense part * broadcast ctx_feats; ACT: per-slot dot-product accumulates
  and the clip+softplus chain via relu(x+10) -> relu(20-x) ->
  softplus(+/-(x-10)); a final PE ones-matmul collapses partitions.
"""

import numpy as np

V = 100000
D = 128
NCTX = 10  # 2*window
NSNS = 6   # 1 pos + 5 neg
K = NCTX + NSNS
B = 131072
NCORES = 8
BCORE = B // NCORES
NBLK_FULL = BCORE // 128
SENSE_OFF = V + 1
TABLE_ROWS_FULL = (V + 1) + 3 * V

SEGW = 32768   # table rows per int16-addressable segment
SBB = 16       # blocks per superblock
SBROWS = SBB * 128
SBPOS = SBROWS * K
MAXCALL = 2048

_cache = {}


# ---------------------------------------------------------------------------
# v4: two-stage 4-queue dma_gather
# ---------------------------------------------------------------------------

def plan_windows(caps):
    """caps (rows, mult of 128) per segment -> [(seg, cap, win_base)], total."""
    wins = []
    pos = 0
    for s, cap in enumerate(caps):
        if cap:
            wins.append((s, cap, pos))
            pos += cap
    return wins, pos


def build_nc_v4(nblk, table_rows, caps, gbufs=8):
    import concourse.bacc as bacc
    import concourse.mybir as mybir
    import concourse.tile as tile
    from concourse.library_config import mlp

    f32 = mybir.dt.float32
    i16 = mybir.dt.int16
    ALU = mybir.AluOpType
    ACTF = mybir.ActivationFunctionType
    AX = mybir.AxisListType

    nsb = nblk // SBB
    wins, stage_rows = plan_windows(caps)
    assert stage_rows % 128 == 0 and stage_rows <= SEGW
    i1cols = stage_rows // 16
    i2cols = SBPOS // 16

    nc = bacc.Bacc(
        "TRN2", target_bir_lowering=False, debug=False, num_swdge_queues=4
    )
    table = nc.dram_tensor("table", [table_rows, D], f32, kind="ExternalInput")
    idx1 = nc.dram_tensor("idx1", [128, nsb * i1cols], i16, kind="ExternalInput")
    idx2 = nc.dram_tensor("idx2", [128, nsb * i2cols], i16, kind="ExternalInput")
    msk = nc.dram_tensor("msk", [128, nblk * NSNS], f32, kind="ExternalInput")
    wb = nc.dram_tensor("wb", [128, NCTX * D], f32, kind="ExternalInput")
    out = nc.dram_tensor("out", [1, 2], f32, kind="ExternalOutput")

    rrq = [0]

    def nextq():
        q = rrq[0]
        rrq[0] = (q + 1) % 4
        return q

    with tile.TileContext(nc) as tc:
        with (
            tc.tile_pool(name="const", bufs=1) as constp,
            tc.tile_pool(name="ipool", bufs=2) as ip,
            tc.tile_pool(name="s1pool", bufs=2) as s1p,
            tc.tile_pool(name="dram", bufs=2, space="DRAM") as dp,
            tc.tile_pool(name="gpool", bufs=gbufs) as gp,
            tc.tile_pool(name="wpool", bufs=2) as wp,
            tc.tile_pool(name="spool", bufs=2) as sp,
            tc.tile_pool(name="small", bufs=2) as smp,
            tc.tile_pool(name="psum", bufs=1, space="PSUM") as psp,
        ):
            nc.gpsimd.load_library(mlp)
            msk_t = constp.tile([128, nblk * NSNS], f32)
            nc.sync.dma_start(out=msk_t[:], in_=msk[:])
            wb_t = constp.tile([128, NCTX * D], f32)
            nc.sync.dma_start(out=wb_t[:], in_=wb[:])

            bufP = constp.tile([128, nblk], f32)
            bufN = constp.tile([128, nblk * 5], f32)
            dummy = constp.tile([128, D], f32)
            ones = constp.tile([128, 1], f32)
            nc.vector.memset(ones[:], 1.0)
            c10 = constp.tile([128, 1], f32)
            nc.vector.memset(c10[:], 10.0)
            c20 = constp.tile([128, 1], f32)
            nc.vector.memset(c20[:], 20.0)
            cm10 = constp.tile([128, 1], f32)
            nc.vector.memset(cm10[:], -10.0)

            for sb in range(nsb):
                idx1_t = ip.tile([128, i1cols], i16, tag="i1")
                nc.sync.dma_start(
                    out=idx1_t[:], in_=idx1[:, sb * i1cols : (sb + 1) * i1cols]
                )
                idx2_t = ip.tile([128, i2cols], i16, tag="i2")
                nc.sync.dma_start(
                    out=idx2_t[:], in_=idx2[:, sb * i2cols : (sb + 1) * i2cols]
                )
                staged = dp.tile([stage_rows, D], f32, tag="staged")
                stagedv = staged[:].rearrange("(p c) d -> p c d", p=128)
                for (s, cap, wbase) in wins:
                    # gather this window in <=MAXCALL chunks into one SBUF
                    # tile, then drain the whole window with one HWDGE DMA
                    # (each partition's share of staging is contiguous)
                    wtile = s1p.tile([128, (cap // 128) * D], f32, tag="s1")
                    left, cbase = cap, 0
                    while left > 0:
                        l = min(MAXCALL, left)
                        nc.gpsimd.dma_gather(
                            wtile[
                                :, (cbase // 128) * D : ((cbase + l) // 128) * D
                            ].rearrange("p (c d) -> p c d", c=l // 128),
                            table[s * SEGW :, :],
                            idx1_t[
                                :, (wbase + cbase) // 16 : (wbase + cbase + l) // 16
                            ],
                            l, l, D,
                            single_packet=False,
                            queue_num=nextq(),
                        )
                        cbase += l
                        left -= l
                    nc.sync.dma_start(
                        out=stagedv[:, wbase // 128 : (wbase + cap) // 128, :],
                        in_=wtile[:].rearrange("p (c d) -> p c d", c=cap // 128),
                    )
                for b2 in range(SBB):
                    b = sb * SBB + b2
                    g = gp.tile([128, K * D], f32, tag="g")
                    nc.gpsimd.dma_gather(
                        g[:].rearrange("p (n d) -> p n d", n=K),
                        staged[:, :],
                        idx2_t[:, b2 * (K * 128 // 16) : (b2 + 1) * (K * 128 // 16)],
                        K * 128, K * 128, D,
                        single_packet=False,
                        queue_num=nextq(),
                    )
                    wprod = wp.tile([128, NCTX * D], f32, tag="wprod")
                    nc.vector.tensor_tensor(
                        out=wprod[:], in0=g[:, : NCTX * D], in1=wb_t[:], op=ALU.mult
                    )
                    F = smp.tile([128, D], f32, tag="F")
                    nc.vector.tensor_reduce(
                        out=F[:],
                        in_=wprod[:].rearrange("p (j d) -> p d j", j=NCTX),
                        axis=AX.X,
                        op=ALU.add,
                    )
                    S = sp.tile([128, NSNS * D], f32, tag="S")
                    nc.vector.tensor_tensor(
                        out=S[:].rearrange("p (n d) -> p n d", n=NSNS),
                        in0=g[:, NCTX * D :].rearrange("p (n d) -> p n d", n=NSNS),
                        in1=F[:].unsqueeze(1).to_broadcast([128, NSNS, D]),
                        op=ALU.mult,
                    )
                    ips = smp.tile([128, NSNS], f32, tag="ips")
                    for n in range(NSNS):
                        nc.scalar.activation(
                            out=dummy[:],
                            in_=S[:, n * D : (n + 1) * D],
                            func=ACTF.Copy,
                            accum_out=ips[:, n : n + 1],
                        )
                    t1 = smp.tile([128, NSNS], f32, tag="t1")
                    nc.scalar.activation(
                        out=t1[:], in_=ips[:], func=ACTF.Relu, bias=c10[:], scale=1.0
                    )
                    u = smp.tile([128, NSNS], f32, tag="u")
                    nc.scalar.activation(
                        out=u[:], in_=t1[:], func=ACTF.Relu, bias=c20[:], scale=-1.0
                    )
                    ep = smp.tile([128, 1], f32, tag="ep")
                    nc.scalar.activation(
                        out=ep[:], in_=u[:, 0:1], func=ACTF.Exp, bias=cm10[:],
                        scale=1.0,
                    )
                    nc.scalar.activation(
                        out=bufP[:, b : b + 1], in_=ep[:], func=ACTF.Ln,
                        bias=1.0, scale=1.0,
                    )
                    en = smp.tile([128, 5], f32, tag="en")
                    nc.scalar.activation(
                        out=en[:], in_=u[:, 1:NSNS], func=ACTF.Exp,
                        bias=c10[:], scale=-1.0,
                    )
                    Ln = smp.tile([128, 5], f32, tag="Ln")
                    nc.scalar.activation(
                        out=Ln[:], in_=en[:], func=ACTF.Ln, bias=1.0, scale=1.0
                    )
                    nc.vector.tensor_tensor(
                        out=bufN[:, b * 5 : (b + 1) * 5],
                        in0=Ln[:],
                        in1=msk_t[:, b * NSNS + 1 : (b + 1) * NSNS],
                        op=ALU.mult,
                    )

            acc2 = constp.tile([128, 2], f32)
            nc.vector.tensor_reduce(
                out=acc2[:, 0:1], in_=bufP[:], axis=AX.X, op=ALU.add
            )
            nc.vector.tensor_reduce(
                out=acc2[:, 1:2], in_=bufN[:], axis=AX.X, op=ALU.add
            )
            ps = psp.tile([1, 2], f32)
            nc.tensor.matmul(out=ps[:], lhsT=ones[:], rhs=acc2[:], start=True, stop=True)
            fin = smp.tile([1, 2], f32, tag="fin")
            nc.vector.tensor_copy(out=fin[:], in_=ps[:])
            nc.sync.dma_start(out=out[:], in_=fin[:])

    nc.compile()
    return nc


def get_nc_v4(nblk, table_rows, caps, gbufs=8):
    key = ("v4", nblk, table_rows, caps, gbufs)
    if key not in _cache:
        _cache[key] = build_nc_v4(nblk, table_rows, caps, gbufs)
    return _cache[key]


def _wrap16(lst):
    """index list (len mult of 16) -> [128, len/16] int16 (16-partition wrap,
    replicated into the 8 groups of 16 partitions)."""
    a = np.asarray(lst, dtype=np.int16).reshape(-1, 16).T
    return np.tile(a, (8, 1))


def host_prep_v4(data, global_W, sense_W, ctx_weight, ncores, nblk):
    """Returns (in_maps, caps, ok); ok=False -> staging overflow (fall back)."""
    data = np.asarray(data)
    b = data.shape[0]
    bcore = b // ncores
    nsb = nblk // SBB

    idx_all = np.empty((b, K), dtype=np.int64)
    idx_all[:, :NCTX] = data[:, :NCTX]
    idx_all[:, NCTX] = data[:, NCTX + 1] + SENSE_OFF
    idx_all[:, NCTX + 1 :] = data[:, NCTX + 2 : NCTX + 7] + SENSE_OFF

    maskv = np.ones((b, K), dtype=bool)
    maskv[:, NCTX + 1 :] = data[:, NCTX + 7 :] != 0

    msk_all = np.empty((b, NSNS), dtype=np.float32)
    msk_all[:, 0] = 1.0
    msk_all[:, 1:] = data[:, NCTX + 7 :].astype(np.float32)

    nseg = int(idx_all.max()) // SEGW + 1

    # pass 1: per (core, sb) unique rows and the max per-segment unique count
    percu = []
    maxcnt = np.zeros(nseg, dtype=np.int64)
    for c in range(ncores):
        for sb in range(nsb):
            r0 = c * bcore + sb * SBROWS
            tbl_f = (
                idx_all[r0 : r0 + SBROWS]
                .reshape(SBB, 128, K)
                .transpose(0, 2, 1)
                .reshape(-1)
            )
            vld_f = (
                maskv[r0 : r0 + SBROWS]
                .reshape(SBB, 128, K)
                .transpose(0, 2, 1)
                .reshape(-1)
            )
            vpos = np.nonzero(vld_f)[0]
            uniq, inv = np.unique(tbl_f[vpos], return_inverse=True)
            percu.append((vpos, uniq, inv, vld_f))
            cnt = np.bincount(uniq // SEGW, minlength=nseg)
            maxcnt = np.maximum(maxcnt, cnt)

    caps = tuple(int(-(-c // 128) * 128) for c in maxcnt)
    wins, stage_rows = plan_windows(caps)
    if stage_rows > SEGW:
        return None, caps, False
    sr128 = stage_rows // 128
    wbase_arr = np.zeros(nseg, dtype=np.int64)
    for (s, cap, wb_) in wins:
        wbase_arr[s] = wb_

    table = np.ascontiguousarray(
        np.concatenate(
            [np.asarray(global_W, np.float32), np.asarray(sense_W, np.float32)],
            axis=0,
        )
    )
    wb = np.ascontiguousarray(
        np.broadcast_to(
            np.asarray(ctx_weight, np.float32).reshape(1, NCTX * D),
            (128, NCTX * D),
        )
    )

    posslot = (
        (np.arange(SBPOS) // (K * 128)) * (K * 128)
        + NCTX * 128
        + (np.arange(SBPOS) % 128)
    )

    in_maps = []
    it = iter(percu)
    for c in range(ncores):
        sl = slice(c * bcore, (c + 1) * bcore)
        msk_c = np.ascontiguousarray(
            msk_all[sl]
            .reshape(nblk, 128, NSNS)
            .transpose(1, 0, 2)
            .reshape(128, nblk * NSNS)
        )
        i1_parts, i2_parts = [], []
        for sb in range(nsb):
            vpos, uniq, inv, vld_f = next(it)
            seg = uniq // SEGW
            seg_starts = np.searchsorted(seg, np.arange(nseg))
            spos = wbase_arr[seg] + (np.arange(len(uniq)) - seg_starts[seg])
            # staged DRAM row of window-relative position e' ((p c) layout,
            # drain tile col = e'//128): (e'%128)*sr128 + wbase//128 + e'//128
            ew = spos - wbase_arr[seg]
            staged_row = (ew % 128) * sr128 + wbase_arr[seg] // 128 + ew // 128
            idx1_flat = np.zeros(stage_rows, dtype=np.int64)
            idx1_flat[spos] = uniq - seg * SEGW
            sp_f = np.zeros(SBPOS, dtype=np.int64)
            sp_f[vpos] = staged_row[inv]
            sp_f = np.where(vld_f, sp_f, sp_f[posslot])
            i1_parts.append(_wrap16(idx1_flat))
            i2_parts.append(_wrap16(sp_f))
        in_maps.append(
            {
                "table": table,
                "idx1": np.ascontiguousarray(np.concatenate(i1_parts, axis=1)),
                "idx2": np.ascontiguousarray(np.concatenate(i2_parts, axis=1)),
                "msk": msk_c,
                "wb": wb,
            }
        )
    return in_maps, caps, True


# ---------------------------------------------------------------------------
# v1 fallback: per-slot vector-indirect gathers (one offset per partition per
# instruction).  Slow but distribution-independent.
# ---------------------------------------------------------------------------

def build_nc(nblk, table_rows):
    import concourse.bacc as bacc
    import concourse.bass as bass
    import concourse.mybir as mybir
    import concourse.tile as tile

    f32 = mybir.dt.float32
    i32 = mybir.dt.int32
    ALU = mybir.AluOpType
    ACTF = mybir.ActivationFunctionType
    AX = mybir.AxisListType

    nc = bacc.Bacc("TRN2", target_bir_lowering=False, debug=False)

    table = nc.dram_tensor("table", [table_rows, D], f32, kind="ExternalInput")
    idx = nc.dram_tensor("idx", [128, nblk * K], i32, kind="ExternalInput")
    msk = nc.dram_tensor("msk", [128, nblk * NSNS], f32, kind="ExternalInput")
    wb = nc.dram_tensor("wb", [128, NCTX * D], f32, kind="ExternalInput")
    out = nc.dram_tensor("out", [1, 2], f32, kind="ExternalOutput")

    with tile.TileContext(nc) as tc:
        with (
            tc.tile_pool(name="const", bufs=1) as constp,
            tc.tile_pool(name="gpool", bufs=4) as gp,
            tc.tile_pool(name="wpool", bufs=2) as wp,
            tc.tile_pool(name="spool", bufs=2) as sp,
            tc.tile_pool(name="small", bufs=2) as smp,
            tc.tile_pool(name="psum", bufs=1, space="PSUM") as psp,
        ):
            idx_t = constp.tile([128, nblk * K], i32)
            nc.sync.dma_start(out=idx_t[:], in_=idx[:])
            msk_t = constp.tile([128, nblk * NSNS], f32)
            nc.sync.dma_start(out=msk_t[:], in_=msk[:])
            wb_t = constp.tile([128, NCTX * D], f32)
            nc.sync.dma_start(out=wb_t[:], in_=wb[:])

            bufP = constp.tile([128, nblk], f32)
            bufN = constp.tile([128, nblk * 5], f32)
            dummy = constp.tile([128, D], f32)
            ones = constp.tile([128, 1], f32)
            nc.vector.memset(ones[:], 1.0)
            c10 = constp.tile([128, 1], f32)
            nc.vector.memset(c10[:], 10.0)
            c20 = constp.tile([128, 1], f32)
            nc.vector.memset(c20[:], 20.0)
            cm10 = constp.tile([128, 1], f32)
            nc.vector.memset(cm10[:], -10.0)

            for b in range(nblk):
                g = gp.tile([128, K * D], f32, tag="g")
                for k in range(K):
                    nc.gpsimd.indirect_dma_start(
                        out=g[:, k * D : (k + 1) * D],
                        out_offset=None,
                        in_=table[:],
                        in_offset=bass.IndirectOffsetOnAxis(
                            ap=idx_t[:, b * K + k : b * K + k + 1], axis=0
                        ),
                    )
                wprod = wp.tile([128, NCTX * D], f32, tag="wprod")
                nc.vector.tensor_tensor(
                    out=wprod[:], in0=g[:, : NCTX * D], in1=wb_t[:], op=ALU.mult
                )
                F = smp.tile([128, D], f32, tag="F")
                nc.vector.tensor_reduce(
                    out=F[:],
                    in_=wprod[:].rearrange("p (j d) -> p d j", j=NCTX),
                    axis=AX.X,
                    op=ALU.add,
                )
                S = sp.tile([128, NSNS * D], f32, tag="S")
                nc.vector.tensor_tensor(
                    out=S[:].rearrange("p (n d) -> p n d", n=NSNS),
                    in0=g[:, NCTX * D :].rearrange("p (n d) -> p n d", n=NSNS),
                    in1=F[:].unsqueeze(1).to_broadcast([128, NSNS, D]),
                    op=ALU.mult,
                )
                ips = smp.tile([128, NSNS], f32, tag="ips")
                for n in range(NSNS):
                    nc.scalar.activation(
                        out=dummy[:],
                        in_=S[:, n * D : (n + 1) * D],
                        func=ACTF.Copy,
                        accum_out=ips[:, n : n + 1],
                    )
                t1 = smp.tile([128, NSNS], f32, tag="t1")
                nc.scalar.activation(
                    out=t1[:], in_=ips[:], func=ACTF.Relu, bias=c10[:], scale=1.0
                )
                u = smp.tile([128, NSNS], f32, tag="u")
                nc.scalar.activation(
                    out=u[:], in_=t1[:], func=ACTF.Relu, bias=c20[:], scale=-1.0
                )
                ep = smp.tile([128, 1], f32, tag="ep")
                nc.scalar.activation(
                    out=ep[:], in_=u[:, 0:1], func=ACTF.Exp, bias=cm10[:], scale=1.0
                )
                nc.scalar.activation(
                    out=bufP[:, b : b + 1], in_=ep[:], func=ACTF.Ln, bias=1.0, scale=1.0
                )
                en = smp.tile([128, 5], f32, tag="en")
                nc.scalar.activation(
                    out=en[:], in_=u[:, 1:NSNS], func=ACTF.Exp, bias=c10[:], scale=-1.0
                )
                Ln = smp.tile([128, 5], f32, tag="Ln")
                nc.scalar.activation(
                    out=Ln[:], in_=en[:], func=ACTF.Ln, bias=1.0, scale=1.0
                )
                nc.vector.tensor_tensor(
                    out=bufN[:, b * 5 : (b + 1) * 5],
                    in0=Ln[:],
                    in1=msk_t[:, b * NSNS + 1 : (b + 1) * NSNS],
                    op=ALU.mult,
                )

            acc2 = constp.tile([128, 2], f32)
            nc.vector.tensor_reduce(
                out=acc2[:, 0:1], in_=bufP[:], axis=AX.X, op=ALU.add
            )
            nc.vector.tensor_reduce(
                out=acc2[:, 1:2], in_=bufN[:], axis=AX.X, op=ALU.add
            )
            ps = psp.tile([1, 2], f32)
            nc.tensor.matmul(out=ps[:], lhsT=ones[:], rhs=acc2[:], start=True, stop=True)
            fin = smp.tile([1, 2], f32, tag="fin")
            nc.vector.tensor_copy(out=fin[:], in_=ps[:])
            nc.sync.dma_start(out=out[:], in_=fin[:])

    nc.compile()
    return nc


def get_nc(nblk, table_rows):
    key = (nblk, table_rows)
    if key not in _cache:
        _cache[key] = build_nc(nblk, table_rows)
    return _cache[key]


def host_prep(data, global_W, sense_W, ctx_weight, ncores, nblk):
    data = np.asarray(data)
    b = data.shape[0]
    bcore = b // ncores
    assert bcore == nblk * 128

    idx_all = np.empty((b, K), dtype=np.int32)
    idx_all[:, :NCTX] = data[:, :NCTX]
    idx_all[:, NCTX] = data[:, NCTX + 1] + SENSE_OFF
    idx_all[:, NCTX + 1 :] = data[:, NCTX + 2 : NCTX + 7] + SENSE_OFF

    msk_all = np.empty((b, NSNS), dtype=np.float32)
    msk_all[:, 0] = 1.0
    msk_all[:, 1:] = data[:, NCTX + 7 :].astype(np.float32)

    table = np.ascontiguousarray(
        np.concatenate([global_W, sense_W], axis=0), dtype=np.float32
    )
    wb = np.ascontiguousarray(
        np.broadcast_to(ctx_weight.reshape(1, NCTX * D), (128, NCTX * D)),
        dtype=np.float32,
    )

    in_maps = []
    for c in range(ncores):
        sl = slice(c * bcore, (c + 1) * bcore)
        idx_c = np.ascontiguousarray(
            idx_all[sl].reshape(nblk, 128, K).transpose(1, 0, 2).reshape(128, nblk * K)
        )
        msk_c = np.ascontiguousarray(
            msk_all[sl]
            .reshape(nblk, 128, NSNS)
            .transpose(1, 0, 2)
            .reshape(128, nblk * NSNS)
        )
        in_maps.append({"table": table, "idx": idx_c, "msk": msk_c, "wb": wb})
    return in_maps


def kernel(data, global_W, sense_W, ctx_weight, window, negative):
    from concourse.bass_utils import run_bass_kernel_spmd

    assert int(window) == 5 and int(negative) == 5

    in_maps, caps, ok = host_prep_v4(
        data, global_W, sense_W, ctx_weight, NCORES, NBLK_FULL
    )
    if ok:
        nc = get_nc_v4(NBLK_FULL, TABLE_ROWS_FULL, caps)
    else:
        nc = get_nc(NBLK_FULL, TABLE_ROWS_FULL)
        in_maps = host_prep(
            np.asarray(data), np.asarray(global_W, np.float32),
            np.asarray(sense_W, np.float32),
            np.asarray(ctx_weight, np.float32), NCORES, NBLK_FULL,
        )
    res = run_bass_kernel_spmd(nc, in_maps, core_ids=list(range(NCORES)))
    outs = np.stack([r["out"][0] for r in res.results])  # [ncores, 2]
    tot = outs.sum(axis=0)
    return (np.float32(tot[0]), np.float32(tot[1]))
